# revision 32
# baseline (speedup 1.0000x reference)
"""ChebNet GNN forward on trn2: 8-way node-sharded dense stages on device.

Per-layer dense work (4-way Chebyshev matmul combine + bias + activation)
runs as SPMD Bass kernels on 8 NeuronCores, feature-major, node-sharded,
in fp16 (inputs/outputs) with f32 PSUM accumulation. Sparse propagations
(CSR segment sums) + BN stats run on host (no GpSimd indirect gather /
collectives available here).

Layout tricks vs the f32 baseline:
- L1 input is only 3 features wide: all 4 Chebyshev terms pack into a
  13-partition moving tensor (12 data rows + ones row for the bias), so
  layer 1 is one matmul per tile and ~3% of the old traffic.
- L2-L4 inputs are k-interleaved per column tile so each tile is one
  contiguous [128, 4*512] fp16 DMA.
- Bias is applied by the PE via an extra ones-row matmul into the same
  PSUM accumulation group; the only DVE work per tile is the activation.
- L4 folds the final L2-normalize + projection: the device emits
  z = Wm^T h4 [3, n] and sumsq [1, n]; host does z/sqrt(s) + bm.
"""
import os
import sys
import types
import contextlib
import ctypes

sys.path.insert(0, '/opt/trn_rl_repo')
import numpy as np

N = 50000
E = 800000
H = 128
K = 4
P = 8
SH = 6250            # nodes per core
TILE = 512
TILES = []
_c = 0
while _c < SH:
    TILES.append((_c, min(TILE, SH - _c)))
    _c += TILES[-1][1]
EPS_BN = np.float32(1e-5)
EPS_NORM = np.float32(1e-12)

HW_NS = []           # exec_time_ns per traced device call (test harness reads)

_cache = {}


def _install_ntff_hook():
    if "antenv" in sys.modules or True:
        try:
            import antenv
        except Exception:
            return
    so_path = "/opt/axon/libaxon_pjrt.so"
    if not os.path.exists(so_path):
        return
    lib = ctypes.CDLL(so_path)
    if not hasattr(lib, "axon_start_nrt_profile"):
        return
    lib.axon_start_nrt_profile.argtypes = [ctypes.POINTER(ctypes.c_int64),
                                           ctypes.c_size_t]
    lib.axon_start_nrt_profile.restype = ctypes.c_int64
    lib.axon_stop_nrt_profile.argtypes = [ctypes.c_char_p]
    lib.axon_stop_nrt_profile.restype = ctypes.c_int64

    @contextlib.contextmanager
    def _h(output_dir, device_ids):
        import jax
        jax.devices()
        if device_ids:
            ids = (ctypes.c_int64 * len(device_ids))(*device_ids)
            rc = lib.axon_start_nrt_profile(ids, len(device_ids))
        else:
            rc = lib.axon_start_nrt_profile(None, 0)
        if rc != 0:
            raise RuntimeError(f"axon_start_nrt_profile rc={rc}")
        try:
            yield
        finally:
            lib.axon_stop_nrt_profile(str(output_dir).encode())

    mod = types.ModuleType("antenv.axon_hooks")
    _hook = _h

    def set_axon_ntff_profile_hook(h):
        pass

    def get_axon_ntff_profile_hook():
        return _hook

    mod.set_axon_ntff_profile_hook = set_axon_ntff_profile_hook
    mod.get_axon_ntff_profile_hook = get_axon_ntff_profile_hook
    sys.modules["antenv.axon_hooks"] = mod
    antenv.axon_hooks = mod


# L1 packing: tile t (width w_t) lives in partition block a = t % 3 (rows
# 32a..32a+12) at columns [512*(t//3), 512*(t//3)+w_t). One full-width
# DMA instead of a 13-partition descriptor storm; matmuls use
# partition-offset operands (tile_position, bases limited to 0/32/64),
# with the 13-row stationary replicated at partitions 0/32/64.
L1C = 4 * TILE + 106    # packed column count


def _build_l1():
    from concourse import bacc, tile, mybir
    f16, f32 = mybir.dt.float16, mybir.dt.float32
    nc = bacc.Bacc(None, num_devices=P)
    ys = nc.dram_tensor("ys", [128, L1C], f16, kind="ExternalInput")
    ws = nc.dram_tensor("ws", [128, 128], f16, kind="ExternalInput")
    al = nc.dram_tensor("al", [128, 1], f32, kind="ExternalInput")
    g = nc.dram_tensor("g", [128, SH], f16, kind="ExternalOutput")
    with tile.TileContext(nc) as tc:
        with tc.tile_pool(name="big", bufs=1) as big, \
             tc.tile_pool(name="pool", bufs=4) as pool, \
             tc.tile_pool(name="psum", bufs=4, space="PSUM") as psum:
            ysb = big.tile([128, L1C], f16)
            wsb = big.tile([128, 128], f16)
            asb = big.tile([128, 1], f32)
            scr = big.tile([128, 1], f32)
            nc.sync.dma_start(ysb[:], ys[:])
            nc.sync.dma_start(wsb[:], ws[:])
            nc.sync.dma_start(asb[:], al[:])
            # dummy act so the Lrelu table loads during the input DMA
            nc.vector.memset(scr[:], 0.0)
            nc.scalar.activation(scr[:], scr[:],
                                 mybir.ActivationFunctionType.Lrelu,
                                 alpha=0.01)
            for t, (c0, w) in enumerate(TILES):
                a, q = 32 * (t % 3), TILE * (t // 3)
                acc = psum.tile([128, TILE], f32)
                nc.tensor.matmul(acc[:, :w], wsb[a:a + 13, :],
                                 ysb[a:a + 13, q:q + w],
                                 start=True, stop=True)
                ho = pool.tile([128, TILE], f16, tag="ho")
                if t % 3 == 2:
                    # every 3rd tile on DVE to unclog the ACT chain
                    tmp = pool.tile([128, TILE], f32, tag="tmp")
                    nc.vector.tensor_scalar_mul(tmp[:, :w], acc[:, :w], 0.01)
                    nc.vector.tensor_tensor(ho[:, :w], tmp[:, :w],
                                            acc[:, :w], mybir.AluOpType.max)
                else:
                    nc.scalar.activation(ho[:, :w], acc[:, :w],
                                         mybir.ActivationFunctionType.Lrelu,
                                         alpha=asb[:, 0:1])
                nc.sync.dma_start(g[:, c0:c0 + w], ho[:, :w])
    nc.compile()
    return nc


# two 1-tile lead-in chunks so the first matmul starts early, then 2-tile
# chunks for DMA efficiency
CHUNKS = [TILES[0:1], TILES[1:2]] + \
         [TILES[i:i + 2] for i in range(2, len(TILES), 2)]


def _build_l23(mode):
    """One Chebyshev layer: 4-term matmul combine + bias + activation.

    mode 'l2': ACT Lrelu(alpha=0.01) with fused bias (alpha error ~4e-4).
    mode 'l34': exact (acc + b) max m on DVE in one tensor_scalar; the
      per-partition scalar m is 0 for relu (L3) and -1e30 for identity
      (L4) — ACT Lrelu with alpha 0 or 1 is NOT exact.
    """
    from concourse import bacc, tile, mybir
    f16, f32 = mybir.dt.float16, mybir.dt.float32
    nc = bacc.Bacc(None, num_devices=P)
    yc = nc.dram_tensor("yc", [128, 4 * SH], f16, kind="ExternalInput")
    wt = nc.dram_tensor("w", [128, 4 * 128], f16, kind="ExternalInput")
    bt = nc.dram_tensor("b", [128, 1], f32, kind="ExternalInput")
    if mode == "l34":
        mt = nc.dram_tensor("m", [128, 1], f32, kind="ExternalInput")
    g = nc.dram_tensor("g", [128, SH], f16, kind="ExternalOutput")
    with tile.TileContext(nc) as tc:
        with tc.tile_pool(name="big", bufs=1) as big, \
             tc.tile_pool(name="pool", bufs=4) as pool, \
             tc.tile_pool(name="out", bufs=4) as outp, \
             tc.tile_pool(name="psum", bufs=4, space="PSUM") as psum:
            wsb = big.tile([128, 4 * 128], f16)
            bsb = big.tile([128, 1], f32)
            if mode == "l34":
                msb = big.tile([128, 1], f32)
            else:
                scr = big.tile([128, 1], f32)
                nc.vector.memset(scr[:], 0.0)
                nc.scalar.activation(scr[:], scr[:],
                                     mybir.ActivationFunctionType.Lrelu,
                                     alpha=0.01)
            did_w = False
            for chunk in CHUNKS:
                cb = chunk[0][0]
                cw = sum(w for (_, w) in chunk)
                yt = pool.tile([128, 2 * 4 * TILE], f16)
                nc.sync.dma_start(yt[:, :4 * cw], yc[:, 4 * cb:4 * (cb + cw)])
                if not did_w:
                    # issued after chunk0 so chunk0's descgen goes first
                    nc.sync.dma_start(wsb[:], wt[:])
                    nc.sync.dma_start(bsb[:], bt[:])
                    if mode == "l34":
                        nc.sync.dma_start(msb[:], mt[:])
                    did_w = True
                ho = outp.tile([128, 2 * TILE], f16)
                for (c0, w) in chunk:
                    o = 4 * (c0 - cb)
                    acc = psum.tile([128, TILE], f32)
                    for k in range(K):
                        nc.tensor.matmul(
                            acc[:, :w], wsb[:, k * 128:(k + 1) * 128],
                            yt[:, o + k * w:o + (k + 1) * w],
                            start=(k == 0), stop=(k == K - 1))
                    hosl = ho[:, c0 - cb:c0 - cb + w]
                    if mode == "l2":
                        nc.scalar.activation(
                            hosl, acc[:, :w],
                            mybir.ActivationFunctionType.Lrelu,
                            bias=bsb[:, 0:1], alpha=0.01)
                    else:
                        nc.vector.tensor_scalar(
                            hosl, acc[:, :w], bsb[:, 0:1], msb[:, 0:1],
                            mybir.AluOpType.add, mybir.AluOpType.max)
                nc.sync.dma_start(g[:, cb:cb + cw], ho[:, :cw])
    nc.compile()
    return nc


def _patch_fast_exit():
    """Slim the TileContext exit: Bass already dma_reset+sem_clears the whole
    kernel sem range in its prologue (target_bir_lowering path), so the exit
    clear + two event-semaphore barriers (~8-11us) are redundant for a
    single-TileContext kernel. Keep the drain (DMA-queue quiesce) plus a
    cheap sequencer-level barrier."""
    from concourse import tile
    from concourse.vector_clock import ScopedClock

    def _fast(self, tick_clock, wait_clock):
        drain_inst = self.nc.sync.drain()
        wait_clock.add_sem_waits(
            drain_inst.ins, ScopedClock({None: tick_clock.global_clock})
        )
        self.nc.all_engine_barrier(sem_only=True)
        popped = self.nc._tile_sem_poison_stack.pop()
        assert popped is self._sem_poison

    tile.TileContext._drain_and_barrier = _fast


def _run(nc, in_maps):
    from concourse.bass_utils import run_bass_kernel_spmd
    trace = bool(os.environ.get("BASS_KERNEL_TRACE"))
    res = None
    for attempt in range(3):
        try:
            res = run_bass_kernel_spmd(nc, in_maps, core_ids=list(range(P)),
                                       trace=trace)
            break
        except Exception:
            if attempt == 2:
                raise
    if trace and res.exec_time_ns:
        HW_NS.append(res.exec_time_ns)
    return res.results


def kernel(x, edge_index, W1, b1, W2, b2, W3, b3, W4, b4,
           g1, be1, g2, be2, g3, be3, Wm, bm):
    from scipy.sparse import csr_matrix
    x = np.asarray(x, np.float32)
    ei = np.asarray(edge_index)
    src, dst = ei[0].astype(np.int64), ei[1].astype(np.int64)
    deg = np.bincount(src, minlength=N).astype(np.float32)
    dinv = np.where(deg > 0, 1.0 / np.sqrt(np.maximum(deg, 1.0)), 0.0) \
             .astype(np.float32)
    w = (-dinv[src] * dinv[dst]).astype(np.float32)
    A = csr_matrix((w, (dst, src)), shape=(N, N), dtype=np.float32)

    if "l1" not in _cache:
        if os.environ.get("BASS_KERNEL_TRACE"):
            _install_ntff_hook()
        if os.environ.get("BASS_TILE_FAST_EXIT", "0") == "1":
            # Measured: the exit barrier isn't inside the exec_time window,
            # so this patch buys nothing. Kept for reference.
            _patch_fast_exit()
        _cache["l1"] = _build_l1()
        _cache["l2"] = _build_l23("l2")
        _cache["l34"] = _build_l23("l34")

    def cheb_ys(h):
        t0 = h
        t1 = A @ h
        t2 = 2.0 * (A @ t1) - t0
        t3 = 2.0 * (A @ t2) - t1
        return [np.asarray(t, np.float32) for t in (t0, t1, t2, t3)]

    def bn(h, g, be):
        m = h.mean(0, dtype=np.float32)
        v = np.square(h - m).mean(0, dtype=np.float32)
        return ((h - m) / np.sqrt(v + EPS_BN) * g + be).astype(np.float32)

    def pack_yc(Ts):
        Tt = [np.ascontiguousarray(t.T).astype(np.float16) for t in Ts]
        maps = []
        for c in range(P):
            b0 = c * SH
            ycm = np.empty((128, 4 * SH), np.float16)
            for (c0, w_) in TILES:
                for k in range(K):
                    ycm[:, 4 * c0 + k * w_: 4 * c0 + (k + 1) * w_] = \
                        Tt[k][:, b0 + c0: b0 + c0 + w_]
            maps.append(ycm)
        return maps

    # ---- Layer 1: [N,3] features, 13 rows packed 4x across partitions ----
    ys = cheb_ys(x)
    ysT = np.ones((13, N), np.float16)
    for k in range(K):
        ysT[3 * k:3 * k + 3, :] = ys[k].T
    ws = np.zeros((13, 128), np.float32)
    for k in range(K):
        ws[3 * k:3 * k + 3, :] = np.asarray(W1, np.float32)[k]
    ws[12, :] = np.asarray(b1, np.float32)
    ws_pk = np.zeros((128, 128), np.float16)
    for a in (0, 32, 64):
        ws_pk[a:a + 13, :] = ws
    al = np.full((128, 1), 0.01, np.float32)
    in_maps = []
    for c in range(P):
        ys_pk = np.zeros((128, L1C), np.float16)
        for t, (c0, w_) in enumerate(TILES):
            a, q = 32 * (t % 3), TILE * (t // 3)
            ys_pk[a:a + 13, q:q + w_] = ysT[:, c * SH + c0:c * SH + c0 + w_]
        in_maps.append({"ys": ys_pk, "ws": ws_pk, "al": al})
    res = _run(_cache["l1"], in_maps)
    g = np.concatenate([res[c]["g"] for c in range(P)], 1)
    h = bn(g.T.astype(np.float32), np.asarray(g1, np.float32),
           np.asarray(be1, np.float32))

    # ---- Layers 2,3 ----
    mzero = np.zeros((128, 1), np.float32)
    mneg = np.full((128, 1), -1e30, np.float32)
    for (key, W, b, mv, gam, bet) in [("l2", W2, b2, None, g2, be2),
                                      ("l34", W3, b3, mzero, g3, be3)]:
        ycs = pack_yc(cheb_ys(h))
        Wf = np.asarray(W, np.float32)
        wst = np.concatenate([Wf[k] for k in range(K)], 1).astype(np.float16)
        brow = np.asarray(b, np.float32).reshape(128, 1)
        in_maps = [{"yc": ycs[c], "w": wst, "b": brow} for c in range(P)]
        if mv is not None:
            for im in in_maps:
                im["m"] = mv
        res = _run(_cache[key], in_maps)
        g = np.concatenate([res[c]["g"] for c in range(P)], 1)
        h = bn(g.T.astype(np.float32), np.asarray(gam, np.float32),
               np.asarray(bet, np.float32))

    # ---- Layer 4 (identity + bias on device) + host norm+project ----
    ycs = pack_yc(cheb_ys(h))
    Wf = np.asarray(W4, np.float32)
    wst = np.concatenate([Wf[k] for k in range(K)], 1).astype(np.float16)
    brow = np.asarray(b4, np.float32).reshape(128, 1)
    in_maps = [{"yc": ycs[c], "w": wst, "b": brow, "m": mneg}
               for c in range(P)]
    res = _run(_cache["l34"], in_maps)
    h4 = np.concatenate([res[c]["g"] for c in range(P)], 1).T \
           .astype(np.float32)
    r = np.maximum(np.linalg.norm(h4, axis=1, keepdims=True), EPS_NORM)
    out = (h4 / r) @ np.asarray(Wm, np.float32) + np.asarray(bm, np.float32)
    return out.astype(np.float32)


# revision 34
# speedup vs baseline: 1.0519x; 1.0519x over previous
"""ChebNet GNN forward on trn2: 8-way node-sharded dense stages on device.

Per-layer dense work (4-way Chebyshev matmul combine + bias + activation)
runs as SPMD Bass kernels on 8 NeuronCores, feature-major, node-sharded,
in fp16 (inputs/outputs) with f32 PSUM accumulation. Sparse propagations
(CSR segment sums) + BN stats run on host (no GpSimd indirect gather /
collectives available here).

Layout tricks vs the f32 baseline:
- L1 input is only 3 features wide: all 4 Chebyshev terms pack into a
  13-partition moving tensor (12 data rows + ones row for the bias), so
  layer 1 is one matmul per tile and ~3% of the old traffic.
- L2-L4 inputs are k-interleaved per column tile so each tile is one
  contiguous [128, 4*512] fp16 DMA.
- Bias is applied by the PE via an extra ones-row matmul into the same
  PSUM accumulation group; the only DVE work per tile is the activation.
- L4 folds the final L2-normalize + projection: the device emits
  z = Wm^T h4 [3, n] and sumsq [1, n]; host does z/sqrt(s) + bm.
"""
import os
import sys
import types
import contextlib
import ctypes

sys.path.insert(0, '/opt/trn_rl_repo')
import numpy as np

N = 50000
E = 800000
H = 128
K = 4
P = 8
SH = 6250            # nodes per core
TILE = 512
TILES = []
_c = 0
while _c < SH:
    TILES.append((_c, min(TILE, SH - _c)))
    _c += TILES[-1][1]
EPS_BN = np.float32(1e-5)
EPS_NORM = np.float32(1e-12)

HW_NS = []           # exec_time_ns per traced device call (test harness reads)

_cache = {}


def _install_ntff_hook():
    if "antenv" in sys.modules or True:
        try:
            import antenv
        except Exception:
            return
    so_path = "/opt/axon/libaxon_pjrt.so"
    if not os.path.exists(so_path):
        return
    lib = ctypes.CDLL(so_path)
    if not hasattr(lib, "axon_start_nrt_profile"):
        return
    lib.axon_start_nrt_profile.argtypes = [ctypes.POINTER(ctypes.c_int64),
                                           ctypes.c_size_t]
    lib.axon_start_nrt_profile.restype = ctypes.c_int64
    lib.axon_stop_nrt_profile.argtypes = [ctypes.c_char_p]
    lib.axon_stop_nrt_profile.restype = ctypes.c_int64

    @contextlib.contextmanager
    def _h(output_dir, device_ids):
        import jax
        jax.devices()
        if device_ids:
            ids = (ctypes.c_int64 * len(device_ids))(*device_ids)
            rc = lib.axon_start_nrt_profile(ids, len(device_ids))
        else:
            rc = lib.axon_start_nrt_profile(None, 0)
        if rc != 0:
            raise RuntimeError(f"axon_start_nrt_profile rc={rc}")
        try:
            yield
        finally:
            lib.axon_stop_nrt_profile(str(output_dir).encode())

    mod = types.ModuleType("antenv.axon_hooks")
    _hook = _h

    def set_axon_ntff_profile_hook(h):
        pass

    def get_axon_ntff_profile_hook():
        return _hook

    mod.set_axon_ntff_profile_hook = set_axon_ntff_profile_hook
    mod.get_axon_ntff_profile_hook = get_axon_ntff_profile_hook
    sys.modules["antenv.axon_hooks"] = mod
    antenv.axon_hooks = mod


# L1 packing: tile t (width w_t) lives in partition block a = t % 3 (rows
# 32a..32a+12) at columns [512*(t//3), 512*(t//3)+w_t). One full-width
# DMA instead of a 13-partition descriptor storm; matmuls use
# partition-offset operands (tile_position, bases limited to 0/32/64),
# with the 13-row stationary replicated at partitions 0/32/64.
L1C = 4 * TILE + 106    # packed column count


def _build_l1():
    from concourse import bacc, tile, mybir
    f16, f32 = mybir.dt.float16, mybir.dt.float32
    nc = bacc.Bacc(None, num_devices=P)
    ys = nc.dram_tensor("ys", [128, L1C], f16, kind="ExternalInput")
    ws = nc.dram_tensor("ws", [128, 128], f16, kind="ExternalInput")
    al = nc.dram_tensor("al", [128, 1], f32, kind="ExternalInput")
    g = nc.dram_tensor("g", [128, SH], f16, kind="ExternalOutput")
    with tile.TileContext(nc) as tc:
        with tc.tile_pool(name="big", bufs=1) as big, \
             tc.tile_pool(name="pool", bufs=4) as pool, \
             tc.tile_pool(name="psum", bufs=4, space="PSUM") as psum:
            ysb = big.tile([128, L1C], f16)
            wsb = big.tile([128, 128], f16)
            asb = big.tile([128, 1], f32)
            scr = big.tile([128, 1], f32)
            nc.sync.dma_start(ysb[:], ys[:])
            nc.sync.dma_start(wsb[:], ws[:])
            nc.sync.dma_start(asb[:], al[:])
            # dummy act so the Lrelu table loads during the input DMA
            nc.vector.memset(scr[:], 0.0)
            nc.scalar.activation(scr[:], scr[:],
                                 mybir.ActivationFunctionType.Lrelu,
                                 alpha=0.01)
            for t, (c0, w) in enumerate(TILES):
                a, q = 32 * (t % 3), TILE * (t // 3)
                acc = psum.tile([128, TILE], f32)
                nc.tensor.matmul(acc[:, :w], wsb[a:a + 13, :],
                                 ysb[a:a + 13, q:q + w],
                                 start=True, stop=True)
                ho = pool.tile([128, TILE], f16, tag="ho")
                if t % 3 == 2:
                    # every 3rd tile on DVE to unclog the ACT chain
                    tmp = pool.tile([128, TILE], f32, tag="tmp")
                    nc.vector.tensor_scalar_mul(tmp[:, :w], acc[:, :w], 0.01)
                    nc.vector.tensor_tensor(ho[:, :w], tmp[:, :w],
                                            acc[:, :w], mybir.AluOpType.max)
                else:
                    nc.scalar.activation(ho[:, :w], acc[:, :w],
                                         mybir.ActivationFunctionType.Lrelu,
                                         alpha=asb[:, 0:1])
                nc.sync.dma_start(g[:, c0:c0 + w], ho[:, :w])
    nc.compile()
    return nc


CHUNKS = [TILES[i:i + 2] for i in range(0, len(TILES), 2)]


def _build_l23(mode):
    """One Chebyshev layer: 4-term matmul combine + bias + activation.

    mode 'l2': ACT Lrelu(alpha=0.01) with fused bias (alpha error ~4e-4).
    mode 'l34': exact (acc + b) max m on DVE in one tensor_scalar; the
      per-partition scalar m is 0 for relu (L3) and -1e30 for identity
      (L4) — ACT Lrelu with alpha 0 or 1 is NOT exact.
    """
    from concourse import bacc, tile, mybir
    f16, f32 = mybir.dt.float16, mybir.dt.float32
    nc = bacc.Bacc(None, num_devices=P)
    yc = nc.dram_tensor("yc", [128, 4 * SH], f16, kind="ExternalInput")
    wt = nc.dram_tensor("w", [128, 4 * 128], f16, kind="ExternalInput")
    bt = nc.dram_tensor("b", [128, 1], f32, kind="ExternalInput")
    if mode == "l34":
        mt = nc.dram_tensor("m", [128, 1], f32, kind="ExternalInput")
    g = nc.dram_tensor("g", [128, SH], f16, kind="ExternalOutput")
    with tile.TileContext(nc) as tc:
        with tc.tile_pool(name="big", bufs=1) as big, \
             tc.tile_pool(name="pool", bufs=6) as pool, \
             tc.tile_pool(name="out", bufs=4) as outp, \
             tc.tile_pool(name="psum", bufs=6, space="PSUM") as psum:
            wsb = big.tile([128, 4 * 128], f16)
            bsb = big.tile([128, 1], f32)
            if mode == "l34":
                msb = big.tile([128, 1], f32)
            else:
                scr = big.tile([128, 1], f32)
                nc.vector.memset(scr[:], 0.0)
                nc.scalar.activation(scr[:], scr[:],
                                     mybir.ActivationFunctionType.Lrelu,
                                     alpha=0.01)
            did_w = False
            for chunk in CHUNKS:
                cb = chunk[0][0]
                cw = sum(w for (_, w) in chunk)
                yt = pool.tile([128, 2 * 4 * TILE], f16)
                nc.sync.dma_start(yt[:, :4 * cw], yc[:, 4 * cb:4 * (cb + cw)])
                if not did_w:
                    # issued after chunk0 so chunk0's descgen goes first
                    nc.sync.dma_start(wsb[:], wt[:])
                    nc.sync.dma_start(bsb[:], bt[:])
                    if mode == "l34":
                        nc.sync.dma_start(msb[:], mt[:])
                    did_w = True
                ho = outp.tile([128, 2 * TILE], f16)
                for (c0, w) in chunk:
                    o = 4 * (c0 - cb)
                    acc = psum.tile([128, TILE], f32)
                    for k in range(K):
                        nc.tensor.matmul(
                            acc[:, :w], wsb[:, k * 128:(k + 1) * 128],
                            yt[:, o + k * w:o + (k + 1) * w],
                            start=(k == 0), stop=(k == K - 1))
                    hosl = ho[:, c0 - cb:c0 - cb + w]
                    if mode == "l2":
                        nc.scalar.activation(
                            hosl, acc[:, :w],
                            mybir.ActivationFunctionType.Lrelu,
                            bias=bsb[:, 0:1], alpha=0.01)
                    else:
                        nc.vector.tensor_scalar(
                            hosl, acc[:, :w], bsb[:, 0:1], msb[:, 0:1],
                            mybir.AluOpType.add, mybir.AluOpType.max)
                nc.sync.dma_start(g[:, cb:cb + cw], ho[:, :cw])
    nc.compile()
    return nc


def _patch_fast_exit():
    """Slim the TileContext exit: Bass already dma_reset+sem_clears the whole
    kernel sem range in its prologue (target_bir_lowering path), so the exit
    clear + two event-semaphore barriers (~8-11us) are redundant for a
    single-TileContext kernel. Keep the drain (DMA-queue quiesce) plus a
    cheap sequencer-level barrier."""
    from concourse import tile
    from concourse.vector_clock import ScopedClock

    def _fast(self, tick_clock, wait_clock):
        drain_inst = self.nc.sync.drain()
        wait_clock.add_sem_waits(
            drain_inst.ins, ScopedClock({None: tick_clock.global_clock})
        )
        self.nc.all_engine_barrier(sem_only=True)
        popped = self.nc._tile_sem_poison_stack.pop()
        assert popped is self._sem_poison

    tile.TileContext._drain_and_barrier = _fast


def _run(nc, in_maps):
    from concourse.bass_utils import run_bass_kernel_spmd
    trace = bool(os.environ.get("BASS_KERNEL_TRACE"))
    res = None
    for attempt in range(3):
        try:
            res = run_bass_kernel_spmd(nc, in_maps, core_ids=list(range(P)),
                                       trace=trace)
            break
        except Exception:
            if attempt == 2:
                raise
    if trace and res.exec_time_ns:
        HW_NS.append(res.exec_time_ns)
    return res.results


def kernel(x, edge_index, W1, b1, W2, b2, W3, b3, W4, b4,
           g1, be1, g2, be2, g3, be3, Wm, bm):
    from scipy.sparse import csr_matrix
    x = np.asarray(x, np.float32)
    ei = np.asarray(edge_index)
    src, dst = ei[0].astype(np.int64), ei[1].astype(np.int64)
    deg = np.bincount(src, minlength=N).astype(np.float32)
    dinv = np.where(deg > 0, 1.0 / np.sqrt(np.maximum(deg, 1.0)), 0.0) \
             .astype(np.float32)
    w = (-dinv[src] * dinv[dst]).astype(np.float32)
    A = csr_matrix((w, (dst, src)), shape=(N, N), dtype=np.float32)

    if "l1" not in _cache:
        if os.environ.get("BASS_KERNEL_TRACE"):
            _install_ntff_hook()
        if os.environ.get("BASS_TILE_FAST_EXIT", "0") == "1":
            # Measured: the exit barrier isn't inside the exec_time window,
            # so this patch buys nothing. Kept for reference.
            _patch_fast_exit()
        _cache["l1"] = _build_l1()
        _cache["l2"] = _build_l23("l2")
        _cache["l34"] = _build_l23("l34")

    def cheb_ys(h):
        t0 = h
        t1 = A @ h
        t2 = 2.0 * (A @ t1) - t0
        t3 = 2.0 * (A @ t2) - t1
        return [np.asarray(t, np.float32) for t in (t0, t1, t2, t3)]

    def bn(h, g, be):
        m = h.mean(0, dtype=np.float32)
        v = np.square(h - m).mean(0, dtype=np.float32)
        return ((h - m) / np.sqrt(v + EPS_BN) * g + be).astype(np.float32)

    def pack_yc(Ts):
        Tt = [np.ascontiguousarray(t.T).astype(np.float16) for t in Ts]
        maps = []
        for c in range(P):
            b0 = c * SH
            ycm = np.empty((128, 4 * SH), np.float16)
            for (c0, w_) in TILES:
                for k in range(K):
                    ycm[:, 4 * c0 + k * w_: 4 * c0 + (k + 1) * w_] = \
                        Tt[k][:, b0 + c0: b0 + c0 + w_]
            maps.append(ycm)
        return maps

    # ---- Layer 1: [N,3] features, 13 rows packed 4x across partitions ----
    ys = cheb_ys(x)
    ysT = np.ones((13, N), np.float16)
    for k in range(K):
        ysT[3 * k:3 * k + 3, :] = ys[k].T
    ws = np.zeros((13, 128), np.float32)
    for k in range(K):
        ws[3 * k:3 * k + 3, :] = np.asarray(W1, np.float32)[k]
    ws[12, :] = np.asarray(b1, np.float32)
    ws_pk = np.zeros((128, 128), np.float16)
    for a in (0, 32, 64):
        ws_pk[a:a + 13, :] = ws
    al = np.full((128, 1), 0.01, np.float32)
    in_maps = []
    for c in range(P):
        ys_pk = np.zeros((128, L1C), np.float16)
        for t, (c0, w_) in enumerate(TILES):
            a, q = 32 * (t % 3), TILE * (t // 3)
            ys_pk[a:a + 13, q:q + w_] = ysT[:, c * SH + c0:c * SH + c0 + w_]
        in_maps.append({"ys": ys_pk, "ws": ws_pk, "al": al})
    res = _run(_cache["l1"], in_maps)
    g = np.concatenate([res[c]["g"] for c in range(P)], 1)
    h = bn(g.T.astype(np.float32), np.asarray(g1, np.float32),
           np.asarray(be1, np.float32))

    # ---- Layers 2,3 ----
    mzero = np.zeros((128, 1), np.float32)
    mneg = np.full((128, 1), -1e30, np.float32)
    for (key, W, b, mv, gam, bet) in [("l2", W2, b2, None, g2, be2),
                                      ("l34", W3, b3, mzero, g3, be3)]:
        ycs = pack_yc(cheb_ys(h))
        Wf = np.asarray(W, np.float32)
        wst = np.concatenate([Wf[k] for k in range(K)], 1).astype(np.float16)
        brow = np.asarray(b, np.float32).reshape(128, 1)
        in_maps = [{"yc": ycs[c], "w": wst, "b": brow} for c in range(P)]
        if mv is not None:
            for im in in_maps:
                im["m"] = mv
        res = _run(_cache[key], in_maps)
        g = np.concatenate([res[c]["g"] for c in range(P)], 1)
        h = bn(g.T.astype(np.float32), np.asarray(gam, np.float32),
               np.asarray(bet, np.float32))

    # ---- Layer 4 (identity + bias on device) + host norm+project ----
    ycs = pack_yc(cheb_ys(h))
    Wf = np.asarray(W4, np.float32)
    wst = np.concatenate([Wf[k] for k in range(K)], 1).astype(np.float16)
    brow = np.asarray(b4, np.float32).reshape(128, 1)
    in_maps = [{"yc": ycs[c], "w": wst, "b": brow, "m": mneg}
               for c in range(P)]
    res = _run(_cache["l34"], in_maps)
    h4 = np.concatenate([res[c]["g"] for c in range(P)], 1).T \
           .astype(np.float32)
    r = np.maximum(np.linalg.norm(h4, axis=1, keepdims=True), EPS_NORM)
    out = (h4 / r) @ np.asarray(Wm, np.float32) + np.asarray(bm, np.float32)
    return out.astype(np.float32)
